# revision 1
# baseline (speedup 1.0000x reference)
"""Trainium2 Bass kernel for nn_EvolutionCrossAttention (B=4, C=128, N=32*64*64).

8-core SPMD, sequence(N)-sharded. The module algebraically reduces to, per (b,h):
     logits[n] = const(b,h) + sum_c M'[b,h,c] * x_raw[b,c,n]
     out       = f( sum_n softmax_n(logits) * x_raw[b,:,n] )
with M' folding q@Wk, the GroupNorm affine and per-group rstd, and f the tiny
O(C^2) output projections (host-side). Softmax constants cancel between
numerator Z*s and denominator Z, and scaled logits have sigma ~= 1, so exp()
needs no max subtraction and per-core partial sums add across cores directly.

Device kernel per core (x shard converted to bf16 on host, SBUF-resident):
  pass A: one DMA pass of x; bn_stats per channel (DVE) + per-group partial
          logits G = mtall_hi.T@x + mtall_lo.T@x (PE, bf16 hi/lo for accuracy),
          G kept in SBUF as fp16.
  tiny 4KB AllReduce of per-channel stats; group rstd folded into R (fp16
          hi/lo) on device.
  pass B: l^T = G.T@[R_hi+R_lo] (PE) ; p = exp(l^T) (ACT, bf16) ;
          x chunks transposed on PE (identity matmul) with a ones column
          appended ; s|Z += p_slice.T @ [x^T|1] accumulated in PSUM.
Host merges (s, Z, stats) across cores and applies GroupNorm affine + Wv/Wo.
"""
import sys

sys.path.insert(0, "/opt/trn_rl_repo")

import numpy as np
import ml_dtypes

import concourse.bass as bass
import concourse.tile as tile
from concourse import mybir
from concourse.bass_utils import run_bass_kernel_spmd

# Problem dims (hardcoded per spec)
B, C = 4, 128
N = 32 * 64 * 64          # 131072
E = 128
NH, HD = 4, 32            # heads, head dim
G, GS = 8, 16             # groupnorm groups, channels per group
EPS = 1e-5
NCORES = 8
NS = N // NCORES          # 16384 per-core columns
TILE = 512
NT = NS // TILE           # 32
CH = 128                  # transpose/matmul chunk
NCH = TILE // CH          # 4
BH = B * NH               # 16

F32 = mybir.dt.float32
BF16 = mybir.dt.bfloat16

_ISA_WAIT_LIMIT = 1


def _split_excess_waits(nc, limit=_ISA_WAIT_LIMIT):
    """This toolchain's codegen accepts only one sem wait per instruction;
    hoist extras onto same-engine nops inserted just before."""
    for bb in nc.main_func.blocks:
        insts = bb.instructions
        i = 0
        while i < len(insts):
            inst = insts[i]
            si = inst.sync_info
            if si is None or not si.on_wait or len(si.on_wait) <= limit:
                i += 1
                continue
            waits = list(si.on_wait)
            si.on_wait = waits[:limit]
            excess = waits[limit:]
            pos = i
            while excess:
                chunk, excess = excess[:limit], excess[limit:]
                nop = mybir.InstNoOp(name=nc.get_next_instruction_name(), ins=[], outs=[])
                nop.engine = inst.engine
                nop.sync_info = mybir.SyncInfo(on_wait=chunk, on_update=[])
                insts.insert(pos, nop)
                pos += 1
                i += 1
            i += 1


def _build_nc(ncores=NCORES, waitfix=True):
    """v2: x stays SBUF-resident; PE transposes; single DMA pass."""
    nc = bass.Bass()
    x = nc.declare_dram_parameter("x", [B, C, NS], BF16, isOutput=False)
    mtall = nc.declare_dram_parameter("mtall", [2, B, C, 128], BF16, isOutput=False)
    pmask = nc.declare_dram_parameter("pmask", [C, BH], F32, isOutput=False)
    gind = nc.declare_dram_parameter("gind", [C, G], F32, isOutput=False)
    ident = nc.declare_dram_parameter("ident", [C, C], BF16, isOutput=False)
    sz = nc.declare_dram_parameter("sz", [B, NH, C + 1], F32, isOutput=True)
    gsout = nc.declare_dram_parameter("gsums", [2 * B, G], F32, isOutput=True)

    FP16 = mybir.dt.float16
    QT = NT // 4                     # tiles per x quarter-part

    with tile.TileContext(nc) as tc:
        from contextlib import ExitStack
        with ExitStack() as ctx:
            consts = ctx.enter_context(tc.tile_pool(name="consts", bufs=1))
            small = ctx.enter_context(tc.tile_pool(name="small", bufs=1))
            xtspool = ctx.enter_context(tc.tile_pool(name="xts", bufs=5))
            ptpool = ctx.enter_context(tc.tile_pool(name="ptp", bufs=3))
            mmp = ctx.enter_context(tc.tile_pool(name="mmp", bufs=2, space="PSUM"))
            xtpp = ctx.enter_context(tc.tile_pool(name="xtpp", bufs=2, space="PSUM"))
            accp = ctx.enter_context(tc.tile_pool(name="accp", bufs=1, space="PSUM"))
            dram = ctx.enter_context(tc.tile_pool(name="dram", bufs=1, space="DRAM"))

            # ---- constants ----
            mtall_sb = consts.tile([C, 2, B, 128], BF16)
            nc.sync.dma_start(mtall_sb[:], mtall[:].rearrange("k b c m -> c k b m"))
            pmask_sb = consts.tile([C, BH], F32)
            nc.sync.dma_start(pmask_sb[:], pmask[:])
            gind_sb = consts.tile([C, G], F32)
            nc.sync.dma_start(gind_sb[:], gind[:])
            ident_sb = consts.tile([C, C], BF16)
            nc.sync.dma_start(ident_sb[:], ident[:])
            ones_f32 = consts.tile([C, B * NCH], BF16, tag="ones4")
            nc.vector.memset(ones_f32[:], 1.0)

            # x resident in SBUF: 16 quarter-batch parts for load/compute overlap
            xparts = [[None] * 4 for _ in range(B)]
            for qq in range(4):
                for b in range(B):
                    xp = consts.tile([C, QT, TILE], BF16, name=f"xsb{b}_{qq}",
                                     tag=f"xsb{b}_{qq}")
                    nc.sync.dma_start(
                        xp[:], x[b, :, qq * QT * TILE:(qq + 1) * QT * TILE])
                    xparts[b][qq] = xp

            def xsl(b, t, lo, hi):
                return xparts[b][t // QT][:, t % QT, lo:hi]

            Gsb = consts.tile([128, NS], FP16, tag="Gbig")
            stat6 = consts.tile([C, B, NT, 6], F32, tag="stat6")

            xts_map = {}

            def emit_transpose(t):
                xts = xtspool.tile([128, B * NCH, CH + 1], BF16, tag="xts")
                for half in range(2):
                    xtp = xtpp.tile([128, 2 * NCH, CH], BF16, tag="xtp")
                    for bb in range(2):
                        b = half * 2 + bb
                        for j in range(NCH):
                            nc.tensor.transpose(xtp[:, bb * NCH + j, :],
                                                xsl(b, t, j * CH, (j + 1) * CH),
                                                ident_sb[:])
                    nc.scalar.copy(
                        xts[:, half * 2 * NCH:(half + 1) * 2 * NCH, 0:CH], xtp[:])
                nc.scalar.mul(xts[:, :, CH:CH + 1], ones_f32[:, 0:B * NCH, None], 1.0)
                xts_map[t] = xts

            # ---- pass A: stats + G ----
            for t in range(NT):
                gp = mmp.tile([128, TILE], F32, tag="mm")
                for b in range(B):
                    nc.vector.bn_stats(stat6[:, b, t, :], xsl(b, t, 0, TILE))
                    nc.tensor.matmul(gp[:], mtall_sb[:, 0, b, :], xsl(b, t, 0, TILE),
                                     start=(b == 0), stop=False)
                    nc.tensor.matmul(gp[:], mtall_sb[:, 1, b, :], xsl(b, t, 0, TILE),
                                     start=False, stop=(b == B - 1))
                nc.scalar.copy(Gsb[:, t * TILE:(t + 1) * TILE], gp[:])

            # Pre-emit the first few tiles' transposes so the scheduler can run
            # them on PE while the stats collective is in flight.
            PREK = 5
            for t in range(PREK):
                emit_transpose(t)

            # ---- stats finish + allreduce + R ----
            mv = small.tile([C, B, 2], F32, tag="mv")
            for b in range(B):
                nc.vector.bn_aggr(mv[:, b, :], stat6[:, b, :, :])
            sq = small.tile([C, B], F32, tag="sq")
            nc.scalar.square(sq[:], mv[:, :, 0])
            ar = small.tile([C, 2 * B], F32, tag="ar")
            nc.vector.tensor_copy(ar[:, 0:B], mv[:, :, 0])
            nc.vector.tensor_add(ar[:, B:2 * B], mv[:, :, 1], sq[:])

            ar_in = dram.tile([C, 2 * B], F32, tag="arin")
            ar_out = dram.tile([C, 2 * B], F32, tag="arout")
            nc.gpsimd.dma_start(ar_in[:], ar[:])
            nc.gpsimd.collective_compute(
                "AllReduce", mybir.AluOpType.add,
                replica_groups=[list(range(ncores))],
                ins=[ar_in.opt()], outs=[ar_out.opt()],
            )
            ar2 = small.tile([C, 2 * B], F32, tag="ar2")
            nc.gpsimd.dma_start(ar2[:], ar_out[:])

            # per-(b,g) sums over the 16 channels of each group, via PE
            gsum_ps = mmp.tile([2 * B, G], F32, tag="mm")
            nc.tensor.matmul(gsum_ps[:], ar2[:], gind_sb[:], start=True, stop=True)
            gs_sb = small.tile([2 * B, G], F32, tag="gs")
            nc.vector.tensor_copy(gs_sb[:], gsum_ps[:])
            nc.gpsimd.dma_start(gsout[:], gs_sb[:])

            # rows 0..3 = sum of means per (b,g); rows 4..7 = sum of E[x^2]
            m2_sb = small.tile([B, G], F32, tag="m2")
            nc.gpsimd.dma_start(m2_sb[:], gs_sb[B:2 * B, :])
            inv = 1.0 / (GS * NCORES)
            mm2 = small.tile([B, G], F32, tag="mm2")
            nc.scalar.mul(mm2[:], m2_sb[:], inv)
            msq = small.tile([B, G], F32, tag="msq")
            nc.scalar.activation(msq[:], gs_sb[0:B, :],
                                 mybir.ActivationFunctionType.Square, scale=inv)
            var_t = small.tile([B, G], F32, tag="var")
            nc.vector.tensor_sub(var_t[:], mm2[:], msq[:])
            eps_sb = small.tile([B, 1], F32, tag="eps")
            nc.vector.memset(eps_sb[:], float(EPS))
            sdt = small.tile([B, G], F32, tag="sdt")
            nc.scalar.activation(sdt[:], var_t[:],
                                 mybir.ActivationFunctionType.Sqrt, bias=eps_sb[:])
            r_t = small.tile([B, G], F32, tag="rt")
            nc.vector.reciprocal(r_t[:], sdt[:])

            r_dram = dram.tile([B, G], F32, tag="rdram")
            nc.gpsimd.dma_start(r_dram[:], r_t[:])
            r128_dram = dram.tile([128], F32, tag="r128")
            rd = r_dram.opt()
            rd_exp = bass.AP(tensor=rd.tensor, offset=rd.offset,
                             ap=[list(rd.ap[0]), [0, NH], list(rd.ap[1])])
            nc.gpsimd.dma_start(
                r128_dram[:].rearrange("(b h g) -> b h g", b=B, h=NH), rd_exp)
            base = small.tile([128, 1], F32, tag="base")
            nc.gpsimd.dma_start(base[:], r128_dram[:, None])
            R_sb = small.tile([128, BH], F32, tag="R")
            nc.vector.tensor_scalar_mul(R_sb[:], pmask_sb[:], base[:, 0:1])
            # fp16 hi/lo split of R (rhs dtype must match fp16 Gsb)
            R_hi = small.tile([128, BH], FP16, tag="Rhi")
            nc.vector.tensor_copy(R_hi[:], R_sb[:])
            R_lo = small.tile([128, BH], FP16, tag="Rlo")
            nc.vector.tensor_sub(R_lo[:], R_sb[:], R_hi[:])

            # ---- pass B ----
            szp = [accp.tile([NH, C + 1], F32, name=f"szp{b}", tag=f"szp{b}")
                   for b in range(B)]
            for t in range(NT):
                lp = mmp.tile([128, NCH * BH], F32, tag="mm")
                for j in range(NCH):
                    gsl = Gsb[:, t * TILE + j * CH: t * TILE + (j + 1) * CH]
                    nc.tensor.matmul(lp[:, j * BH:(j + 1) * BH], gsl, R_hi[:],
                                     start=True, stop=False)
                    nc.tensor.matmul(lp[:, j * BH:(j + 1) * BH], gsl, R_lo[:],
                                     start=False, stop=True)
                pt = ptpool.tile([128, NCH * BH], BF16, tag="pt")
                nc.scalar.activation(pt[:], lp[:], mybir.ActivationFunctionType.Exp)
                if t not in xts_map:
                    emit_transpose(t)
                xts = xts_map.pop(t)
                for b in range(B):
                    for j in range(NCH):
                        pslice = pt[:, j * BH + NH * b: j * BH + NH * b + NH]
                        first = (t == 0 and j == 0)
                        last = (t == NT - 1 and j == NCH - 1)
                        nc.tensor.matmul(szp[b][:], pslice,
                                         xts[:, b * NCH + j, :],
                                         start=first, stop=last)

            for b in range(B):
                ssb = small.tile([NH, C + 1], F32, name=f"ssb{b}", tag=f"ssb{b}")
                nc.vector.tensor_copy(ssb[:], szp[b][:])
                nc.gpsimd.dma_start(sz[b], ssb[:])

    if waitfix:
        _split_excess_waits(nc)
    return nc


_NC_CACHE = {}


def _get_nc():
    if "nc" not in _NC_CACHE:
        _NC_CACHE["nc"] = _build_nc()
    return _NC_CACHE["nc"]


def _host_prep(diff_spatial, evolution_feat, ln_g, ln_b, gn_g, Wq, bq, Wk, bk):
    """Everything O(C^2): layernorm, q, fold q@Wk with GN affine + attn scale."""
    e = evolution_feat.astype(np.float64)
    mu = e.mean(axis=-1, keepdims=True)
    var = e.var(axis=-1, keepdims=True)
    e = (e - mu) / np.sqrt(var + EPS) * ln_g.astype(np.float64) + ln_b.astype(np.float64)
    q = e @ Wq.T.astype(np.float64) + bq.astype(np.float64)      # (B, C)
    q = q.reshape(B, NH, HD)
    # M[b,h,c] = sum_d q[b,h,d] Wk[h*HD+d, c]
    Wkr = Wk.astype(np.float64).reshape(NH, HD, C)
    M = np.einsum("bhd,hdc->bhc", q, Wkr)
    Mfold = M * gn_g.astype(np.float64)[None, None, :] * (HD ** -0.5)

    cg = np.arange(C) // GS                                       # channel -> group
    # mtall[b, c, p] for p = b'*32 + h*8 + g, masked to b'==b and g==cg[c]
    mtall = np.zeros((B, C, 128), np.float64)
    for b in range(B):
        for h in range(NH):
            for g in range(G):
                p = b * 32 + h * 8 + g
                sel = cg == g
                mtall[b, sel, p] = Mfold[b, h, sel]
    # hi/lo bf16 split: two accumulating matmuls recover ~16 mantissa bits
    mt_hi = mtall.astype(ml_dtypes.bfloat16)
    mt_lo = (mtall - mt_hi.astype(np.float64)).astype(ml_dtypes.bfloat16)
    mt2 = np.stack([mt_hi, mt_lo], axis=0)                        # (2, B, C, 128)
    pmask = np.zeros((128, BH), np.float32)
    for b in range(B):
        for h in range(NH):
            for g in range(G):
                pmask[b * 32 + h * 8 + g, b * NH + h] = 1.0
    gindm = (cg[:, None] == np.arange(G)[None, :]).astype(np.float32)
    return q, mt2, pmask, gindm


def kernel(diff_spatial, evolution_feat, ln_g, ln_b, gn_g, gn_b,
           Wq, bq, Wk, bk, Wv, bv, Wo, bo):
    nc = _get_nc()
    xfull = np.asarray(diff_spatial, np.float32).reshape(B, C, N)
    x_bf = xfull.astype(ml_dtypes.bfloat16)

    q, mtall, pmask, gindm = _host_prep(
        np.asarray(diff_spatial, np.float32), np.asarray(evolution_feat, np.float32),
        np.asarray(ln_g, np.float32), np.asarray(ln_b, np.float32),
        np.asarray(gn_g, np.float32), np.asarray(Wq, np.float32),
        np.asarray(bq, np.float32), np.asarray(Wk, np.float32),
        np.asarray(bk, np.float32))

    identv = np.eye(C, dtype=np.float32).astype(ml_dtypes.bfloat16)
    in_maps = []
    for i in range(NCORES):
        in_maps.append({
            "x": np.ascontiguousarray(x_bf[:, :, i * NS:(i + 1) * NS]),
            "mtall": mtall,
            "pmask": pmask,
            "gind": gindm,
            "ident": identv,
        })
    res = run_bass_kernel_spmd(nc, in_maps, list(range(NCORES)))
    return _host_finish(res.results, gn_g, gn_b, Wv, bv, Wo, bo)


def _host_finish(results, gn_g, gn_b, Wv, bv, Wo, bo):
    gs = results[0]["gsums"].astype(np.float64)                 # (2B, G)
    mean_g = gs[0:B, :] / (GS * NCORES)                          # (B, G)
    ex2_g = gs[B:2 * B, :] / (GS * NCORES)
    var_g = ex2_g - mean_g ** 2
    r_g = 1.0 / np.sqrt(var_g + EPS)

    s_tot = np.zeros((B, NH, C), np.float64)
    z_tot = np.zeros((B, NH), np.float64)
    for r in results:
        szv = r["sz"].astype(np.float64)                        # (B, NH, C+1)
        s_tot += szv[:, :, 0:C]
        z_tot += szv[:, :, C]

    cg = np.arange(C) // GS
    a = r_g[:, cg] * np.asarray(gn_g, np.float64)[None, :]       # (B, C)
    d = np.asarray(gn_b, np.float64)[None, :] - mean_g[:, cg] * a
    y = a[:, None, :] * (s_tot / z_tot[:, :, None]) + d[:, None, :]   # (B, NH, C)

    Wvr = np.asarray(Wv, np.float64).reshape(NH, HD, C)
    o1 = np.einsum("hdc,bhc->bhd", Wvr, y).reshape(B, C) + np.asarray(bv, np.float64)
    out = o1 @ np.asarray(Wo, np.float64).T + np.asarray(bo, np.float64)
    return out.astype(np.float32)



# revision 5
# speedup vs baseline: 6.2403x; 6.2403x over previous
"""Trainium2 Bass kernel for nn_EvolutionCrossAttention (B=4, C=128, N=32*64*64).

8-core SPMD, sequence(N)-sharded, collective-free. The module reduces to,
per (b,h):  logits[n] = sum_c A[b,h,c] * x[b,c,n]   (A folds q@Wk, the GN
affine, per-group rstd and the attn scale; the GN mean term is a per-(b,h)
constant that cancels in softmax), then
            out = f( sum_n softmax_n(logits) * x[b,:,n] )
with f the O(C^2) output-side projections. GroupNorm statistics and all
O(C^2) algebra run on host in fp64; the device only does the O(C*N) work.

Device kernel per core (x shard as fp8-e3m4, 8 MiB -> DMA-roofline bound):
  per 256-column chunk of x (SBUF-resident, [C, 256] fp8):
    T: PE transpose of the chunk viewed as [C, 128] bf16 pairs -> PSUM,
       batches of 8 chunks copied to SBUF on DVE/ACT (xts).
    L: 4 matmuls with the fp8 chunk as stationary (even/odd pair-slot view,
       A_hi/A_lo bf16 moving, 4 output cols each) -> logits PSUM.
    exp (ACT, batched over 32 chunks): p = exp(l - 2.5) as fp8-e4m3.
    S: 2 matmuls, xts even/odd fp8 view stationary, p moving -> s[C, H] PSUM.
    Z: 1 matmul per 128 p-columns against a ones vector -> Z partials.
Host merges (s, Z) partials across cores and applies the GN affine + Wv/Wo.
"""
import sys

sys.path.insert(0, "/opt/trn_rl_repo")

import numpy as np
import ml_dtypes

import concourse.bass as bass
import concourse.tile as tile
from concourse import mybir
from concourse.bass_utils import run_bass_kernel_spmd

# Problem dims (hardcoded per spec)
B, C = 4, 128
N = 32 * 64 * 64          # 131072
E = 128
NH, HD = 4, 32            # heads, head dim
G, GS = 8, 16             # groupnorm groups, channels per group
EPS = 1e-5
NCORES = 8
NS = N // NCORES          # 16384 per-core columns
CH = 256                  # x columns per chunk (= 128 bf16 pairs)
CHP = CH // 2             # 128
NCHUNK = B * NS // CH     # 256 chunks per core
KG = 32                   # chunks per exp group (one b spans 2 groups)
NGRP = NCHUNK // KG       # 8
DMB = 4096                # fp8 columns per x DMA block (16 chunks)
NDMA = B * NS // DMB      # 16
TB = 8                    # chunks per transpose-PSUM batch / copy
SHIFT = -2.5              # softmax-invariant logit shift keeping exp in e4m3

F32 = mybir.dt.float32
BF16 = mybir.dt.bfloat16
FP8X = mybir.dt.float8e3   # e3m4 for x (|x| < 15.5, 4 mantissa bits)
FP8P = mybir.dt.float8e4   # e4m3 for p (range to 448)

_ISA_WAIT_LIMIT = 1


def _split_excess_waits(nc, limit=_ISA_WAIT_LIMIT):
    """This toolchain's codegen accepts only one sem wait per instruction;
    hoist extras onto same-engine nops inserted just before."""
    for bb in nc.main_func.blocks:
        insts = bb.instructions
        i = 0
        while i < len(insts):
            inst = insts[i]
            si = inst.sync_info
            if si is None or not si.on_wait or len(si.on_wait) <= limit:
                i += 1
                continue
            waits = list(si.on_wait)
            si.on_wait = waits[:limit]
            excess = waits[limit:]
            pos = i
            while excess:
                chunk, excess = excess[:limit], excess[limit:]
                nop = mybir.InstNoOp(name=nc.get_next_instruction_name(), ins=[], outs=[])
                nop.engine = inst.engine
                nop.sync_info = mybir.SyncInfo(on_wait=chunk, on_update=[])
                insts.insert(pos, nop)
                pos += 1
                i += 1
            i += 1


def _build_nc(ncores=NCORES, waitfix=True):
    nc = bass.Bass()
    x = nc.declare_dram_parameter("x", [C, B * NS], FP8X, isOutput=False)
    aw = nc.declare_dram_parameter("aw", [C, B, 2, NH], BF16, isOutput=False)
    ident = nc.declare_dram_parameter("ident", [C, C], BF16, isOutput=False)
    szout = nc.declare_dram_parameter("szout", [C, B * 5], F32, isOutput=True)

    with tile.TileContext(nc) as tc:
        from contextlib import ExitStack
        with ExitStack() as ctx:
            consts = ctx.enter_context(tc.tile_pool(name="consts", bufs=1))
            xpool = ctx.enter_context(tc.tile_pool(name="xp", bufs=1))
            xtspool = ctx.enter_context(tc.tile_pool(name="xts", bufs=1))
            ppool = ctx.enter_context(tc.tile_pool(name="pp", bufs=1))
            lpp = ctx.enter_context(tc.tile_pool(name="lpp", bufs=2, space="PSUM"))
            xtpp = ctx.enter_context(tc.tile_pool(name="xtpp", bufs=3, space="PSUM"))
            accp = ctx.enter_context(tc.tile_pool(name="accp", bufs=1, space="PSUM"))

            # ---- small consts (SWDGE so the HWDGE queue is free for x) ----
            aw_sb = consts.tile([C, B, 2, NH], BF16)
            nc.gpsimd.dma_start(aw_sb[:], aw[:])
            ident_sb = consts.tile([C, C], BF16)
            nc.gpsimd.dma_start(ident_sb[:], ident[:])
            bias_sb = consts.tile([C, 1], F32, tag="bias")
            nc.vector.memset(bias_sb[:], float(SHIFT))
            ones8 = consts.tile([C, 1], FP8P, tag="ones8")
            nc.vector.memset(ones8[:], 1.0)

            # ---- x: 16 block tiles, one DMA each ----
            xblk = []
            for i in range(NDMA):
                xb = xpool.tile([C, DMB], FP8X, name=f"xb{i}", tag=f"xb{i}")
                nc.sync.dma_start(xb[:], x[:, i * DMB:(i + 1) * DMB])
                xblk.append(xb)

            def xchunk(ch):
                """(even, odd) fp8 APs [C, CHP] for chunk ch + bf16-pair view."""
                blk, off = divmod(ch * CH, DMB)
                t = xblk[blk]
                even = t[:, off:off + CH:2]
                odd = t[:, off + 1:off + CH:2]
                pair = t[:].bitcast(BF16)[:, off // 2:(off + CH) // 2]
                return (even, odd), pair

            # transposed x (bf16-pair layout), one tile per TB-chunk batch
            NBATCH = NCHUNK // TB  # 32
            xts = [xtspool.tile([C, TB * CHP], BF16, name=f"xt{i}", tag=f"xt{i}")
                   for i in range(NBATCH)]

            szp_all = accp.tile([C, B * NH], F32, tag="szp")
            zp_all = accp.tile([C, B], F32, tag="zp")
            szp = [szp_all[:, b * NH:(b + 1) * NH] for b in range(B)]
            zp = [zp_all[:, b:b + 1] for b in range(B)]

            p_tiles = {}

            def emit_group_TL(g):
                """Transposes + logit matmuls + exp for chunk group g."""
                b = g // 2
                lp = lpp.tile([C, KG * 2 * NH], F32, tag="lp")
                for jb in range(KG // TB):          # 4 TB-batches per group
                    bi = g * (KG // TB) + jb        # global batch index
                    xtp = xtpp.tile([C, TB * CHP], BF16, tag="xtp")
                    for j in range(TB):
                        ch = g * KG + jb * TB + j
                        eo, pair = xchunk(ch)
                        nc.tensor.transpose(
                            xtp[:, j * CHP:(j + 1) * CHP], pair, ident_sb[:])
                        jj = jb * TB + j            # chunk index within group
                        for par in (0, 1):
                            col = jj * 2 * NH + par * NH
                            for hl in (0, 1):
                                nc.tensor.matmul(
                                    lp[:, col:col + NH],
                                    eo[par], aw_sb[:, b, hl, :],
                                    start=(hl == 0), stop=(hl == 1))
                    # PSUM -> SBUF copy of the transposed batch (DVE/ACT mix)
                    if jb % 4 == 3:
                        nc.scalar.copy(xts[bi][:], xtp[:])
                    else:
                        nc.vector.tensor_copy(xts[bi][:], xtp[:])
                pt = ppool.tile([C, KG * 2 * NH], FP8P, name=f"pt{g}", tag=f"pt{g}")
                nc.scalar.activation(pt[:], lp[:],
                                     mybir.ActivationFunctionType.Exp,
                                     bias=bias_sb[:])
                p_tiles[g] = pt

            def emit_group_S(g):
                """Weighted-sum + Z matmuls for chunk group g."""
                b = g // 2
                pt = p_tiles[g]
                for jj in range(KG):
                    ch = g * KG + jj
                    bi, j = divmod(ch, TB)
                    x8t = xts[bi][:].bitcast(FP8X)
                    base = j * CH
                    for par in (0, 1):
                        first = (g == 2 * b and jj == 0 and par == 0)
                        last = (g == 2 * b + 1 and jj == KG - 1 and par == 1)
                        nc.tensor.matmul(
                            szp[b][:], x8t[:, base + par:base + CH:2],
                            pt[:, jj * 2 * NH + par * NH: jj * 2 * NH + (par + 1) * NH],
                            start=first, stop=last)
                for half in range(2):
                    nc.tensor.matmul(
                        zp[b][:], pt[:, half * C:(half + 1) * C], ones8[:],
                        start=(g == 2 * b and half == 0),
                        stop=(g == 2 * b + 1 and half == 1))

            emit_group_TL(0)
            emit_group_TL(1)
            for g in range(NGRP):
                if g + 2 < NGRP:
                    emit_group_TL(g + 2)
                emit_group_S(g)

            outsb = consts.tile([C, B * 5], F32, tag="outsb")
            for b in range(B):
                nc.vector.tensor_copy(outsb[:, b * 5:b * 5 + NH], szp[b][:])
                nc.vector.tensor_copy(outsb[:, b * 5 + NH:b * 5 + 5], zp[b][:])
            nc.sync.dma_start(szout[:], outsb[:])

    if waitfix:
        _split_excess_waits(nc)
    return nc


_NC_CACHE = {}


def _get_nc():
    if "nc" not in _NC_CACHE:
        _NC_CACHE["nc"] = _build_nc()
    return _NC_CACHE["nc"]


def _host_prep(diff_spatial, evolution_feat, ln_g, ln_b, gn_g, Wq, bq, Wk):
    """Exact (fp64) GroupNorm stats + folded logit coefficients A, split
    into bf16 hi/lo planes. Also the fp8 x in [C, B*N] layout, zero bytes
    dithered to the smallest denormal so bf16-pair views stay normal."""
    xf = diff_spatial.reshape(B, C, N)
    xg = xf.reshape(B, G, GS, N)
    mu = xg.mean(axis=(2, 3), dtype=np.float64)           # (B, G)
    ex2 = np.einsum("bgcn,bgcn->bg", xg, xg, dtype=np.float64) / (GS * N)
    var = ex2 - mu * mu
    rstd = 1.0 / np.sqrt(var + EPS)                        # (B, G)

    e = evolution_feat.astype(np.float64)
    emu = e.mean(axis=-1, keepdims=True)
    evar = e.var(axis=-1, keepdims=True)
    e = (e - emu) / np.sqrt(evar + EPS) * ln_g.astype(np.float64) + ln_b.astype(np.float64)
    q = e @ Wq.T.astype(np.float64) + bq.astype(np.float64)
    q = q.reshape(B, NH, HD)
    M = np.einsum("bhd,hdc->bhc", q, Wk.astype(np.float64).reshape(NH, HD, C))
    cg = np.arange(C) // GS
    A = (M * gn_g.astype(np.float64)[None, None, :] * (HD ** -0.5)
         * rstd[:, cg][:, None, :])                        # (B, NH, C)

    A_hi = A.astype(ml_dtypes.bfloat16)
    A_lo = (A - A_hi.astype(np.float64)).astype(ml_dtypes.bfloat16)
    aw = np.empty((C, B, 2, NH), ml_dtypes.bfloat16)
    aw[:, :, 0, :] = A_hi.transpose(2, 0, 1)
    aw[:, :, 1, :] = A_lo.transpose(2, 0, 1)

    x8 = np.ascontiguousarray(xf.transpose(1, 0, 2)).astype(ml_dtypes.float8_e3m4)
    v = x8.view(np.uint8)
    zero = (v & 0x7F) == 0
    v[zero] |= 1                                           # +-min denormal

    return x8, aw, mu, rstd


def kernel(diff_spatial, evolution_feat, ln_g, ln_b, gn_g, gn_b,
           Wq, bq, Wk, bk, Wv, bv, Wo, bo):
    nc = _get_nc()
    x8, aw, mu, rstd = _host_prep(
        np.asarray(diff_spatial, np.float32).reshape(B, C, N),
        np.asarray(evolution_feat, np.float32),
        np.asarray(ln_g, np.float32), np.asarray(ln_b, np.float32),
        np.asarray(gn_g, np.float32), np.asarray(Wq, np.float32),
        np.asarray(bq, np.float32), np.asarray(Wk, np.float32))

    identv = np.eye(C, dtype=np.float32).astype(ml_dtypes.bfloat16)
    in_maps = []
    for i in range(NCORES):
        xc = np.ascontiguousarray(x8[:, :, i * NS:(i + 1) * NS]).reshape(C, B * NS)
        in_maps.append({"x": xc, "aw": aw, "ident": identv})
    res = run_bass_kernel_spmd(nc, in_maps, list(range(NCORES)))
    return _host_finish(res.results, mu, rstd, gn_g, gn_b, Wv, bv, Wo, bo)


def _host_finish(results, mu, rstd, gn_g, gn_b, Wv, bv, Wo, bo):
    s_tot = np.zeros((B, NH, C), np.float64)
    z_tot = np.zeros((B, NH), np.float64)
    rr = np.arange(C)
    for r in results:
        o = r["szout"].astype(np.float64)                  # (C, B*5)
        for b in range(B):
            s_tot[b] += o[:, b * 5:b * 5 + NH].T           # (NH, C)
            zcol = o[:, b * 5 + NH]
            for h in range(NH):
                z_tot[b, h] += zcol[rr % NH == h].sum()

    cg = np.arange(C) // GS
    a = rstd[:, cg] * np.asarray(gn_g, np.float64)[None, :]
    d = np.asarray(gn_b, np.float64)[None, :] - mu[:, cg] * a
    y = a[:, None, :] * (s_tot / z_tot[:, :, None]) + d[:, None, :]

    Wvr = np.asarray(Wv, np.float64).reshape(NH, HD, C)
    o1 = np.einsum("hdc,bhc->bhd", Wvr, y).reshape(B, C) + np.asarray(bv, np.float64)
    out = o1 @ np.asarray(Wo, np.float64).T + np.asarray(bo, np.float64)
    return out.astype(np.float32)


# revision 8
# speedup vs baseline: 7.5993x; 1.2178x over previous
"""Trainium2 Bass kernel for nn_EvolutionCrossAttention (B=4, C=128, N=32*64*64).

8-core SPMD, sequence(N)-sharded, collective-free. The module reduces to,
per (b,h):  logits[n] = sum_c A[b,h,c] * x[b,c,n]   (A folds q@Wk, the GN
affine, per-group rstd and the attn scale; the GN mean term is a per-(b,h)
constant that cancels in softmax), then
            out = f( sum_n softmax_n(logits) * x[b,:,n] )
with f the O(C^2) output-side projections. GroupNorm statistics and all
O(C^2) algebra run on host in fp64; the device only does the O(C*N) work.

Device kernel per core (x shard as fp8-e3m4, 8 MiB -> DMA-roofline bound):
  per 256-column chunk of x (SBUF-resident, [C, 256] fp8):
    T: PE transpose of the chunk viewed as [C, 128] bf16 pairs -> PSUM,
       batches of 8 chunks copied to SBUF on DVE/ACT (xts).
    L: 4 matmuls with the fp8 chunk as stationary (even/odd pair-slot view,
       A_hi/A_lo bf16 moving, 4 output cols each) -> logits PSUM.
    exp (ACT, batched over 32 chunks): p = exp(l - 2.5) as fp8-e4m3.
    S: 2 matmuls, xts even/odd fp8 view stationary, p moving -> s[C, H] PSUM.
    Z: 1 matmul per 128 p-columns against a ones vector -> Z partials.
Host merges (s, Z) partials across cores and applies the GN affine + Wv/Wo.
"""
import sys

sys.path.insert(0, "/opt/trn_rl_repo")

import numpy as np
import ml_dtypes

import concourse.bass as bass
import concourse.tile as tile
from concourse import mybir
from concourse.bass_utils import run_bass_kernel_spmd

# Problem dims (hardcoded per spec)
B, C = 4, 128
N = 32 * 64 * 64          # 131072
E = 128
NH, HD = 4, 32            # heads, head dim
G, GS = 8, 16             # groupnorm groups, channels per group
EPS = 1e-5
NCORES = 8
NS = N // NCORES          # 16384 per-core columns
CH = 256                  # x columns per chunk (= 128 bf16 pairs)
CHP = CH // 2             # 128
NCHUNK = B * NS // CH     # 256 chunks per core
KG = 16                   # chunks per exp group (one b spans 4 groups)
NGRP = NCHUNK // KG       # 16
DMB = 4096                # fp8 columns per x DMA block (16 chunks)
NDMA = B * NS // DMB      # 16
TB = 8                    # chunks per transpose-PSUM batch / copy
SHIFT = -2.5              # softmax-invariant logit shift keeping exp in e4m3

F32 = mybir.dt.float32
BF16 = mybir.dt.bfloat16
FP8X = mybir.dt.float8e3   # e3m4 for x (|x| < 15.5, 4 mantissa bits)
FP8P = mybir.dt.float8e4   # e4m3 for p (range to 448)

_ISA_WAIT_LIMIT = 1


def _split_excess_waits(nc, limit=_ISA_WAIT_LIMIT):
    """This toolchain's codegen accepts only one sem wait per instruction;
    hoist extras onto same-engine nops inserted just before."""
    for bb in nc.main_func.blocks:
        insts = bb.instructions
        i = 0
        while i < len(insts):
            inst = insts[i]
            si = inst.sync_info
            if si is None or not si.on_wait or len(si.on_wait) <= limit:
                i += 1
                continue
            waits = list(si.on_wait)
            si.on_wait = waits[:limit]
            excess = waits[limit:]
            pos = i
            while excess:
                chunk, excess = excess[:limit], excess[limit:]
                nop = mybir.InstNoOp(name=nc.get_next_instruction_name(), ins=[], outs=[])
                nop.engine = inst.engine
                nop.sync_info = mybir.SyncInfo(on_wait=chunk, on_update=[])
                insts.insert(pos, nop)
                pos += 1
                i += 1
            i += 1


def _build_nc(ncores=NCORES, waitfix=True):
    nc = bass.Bass()
    x = nc.declare_dram_parameter("x", [C, B * NS], FP8X, isOutput=False)
    aw = nc.declare_dram_parameter("aw", [C, B, 2, NH], BF16, isOutput=False)
    ident = nc.declare_dram_parameter("ident", [C, C], BF16, isOutput=False)
    szout = nc.declare_dram_parameter("szout", [C, B * 5], F32, isOutput=True)

    with tile.TileContext(nc) as tc:
        from contextlib import ExitStack
        with ExitStack() as ctx:
            consts = ctx.enter_context(tc.tile_pool(name="consts", bufs=1))
            xpool = ctx.enter_context(tc.tile_pool(name="xp", bufs=1))
            xtspool = ctx.enter_context(tc.tile_pool(name="xts", bufs=1))
            ppool = ctx.enter_context(tc.tile_pool(name="pp", bufs=1))
            lpp = ctx.enter_context(tc.tile_pool(name="lpp", bufs=2, space="PSUM"))
            xtpp = ctx.enter_context(tc.tile_pool(name="xtpp", bufs=3, space="PSUM"))
            accp = ctx.enter_context(tc.tile_pool(name="accp", bufs=1, space="PSUM"))

            # ---- small consts (SWDGE so the HWDGE queue is free for x) ----
            aw_sb = consts.tile([C, B, 2, NH], BF16)
            nc.gpsimd.dma_start(aw_sb[:], aw[:])
            ident_sb = consts.tile([C, C], BF16)
            nc.gpsimd.dma_start(ident_sb[:], ident[:])
            bias_sb = consts.tile([C, 1], F32, tag="bias")
            nc.vector.memset(bias_sb[:], float(SHIFT))
            ones8 = consts.tile([C, 1], FP8P, tag="ones8")
            nc.vector.memset(ones8[:], 1.0)

            # ---- x: 16 block tiles, one DMA each, alternating HWDGE (SP)
            # and SWDGE (Pool) queues so descriptor generation overlaps ----
            xblk = []
            for i in range(NDMA):
                xb = xpool.tile([C, DMB], FP8X, name=f"xb{i}", tag=f"xb{i}")
                eng = nc.sync if i % 2 == 0 else nc.gpsimd
                eng.dma_start(xb[:], x[:, i * DMB:(i + 1) * DMB])
                xblk.append(xb)

            def xchunk(ch):
                """(even, odd) fp8 APs [C, CHP] for chunk ch + bf16-pair view."""
                blk, off = divmod(ch * CH, DMB)
                t = xblk[blk]
                even = t[:, off:off + CH:2]
                odd = t[:, off + 1:off + CH:2]
                pair = t[:].bitcast(BF16)[:, off // 2:(off + CH) // 2]
                return (even, odd), pair

            # transposed x (bf16-pair layout), one tile per TB-chunk batch
            NBATCH = NCHUNK // TB  # 32
            xts = [xtspool.tile([C, TB * CHP], BF16, name=f"xt{i}", tag=f"xt{i}")
                   for i in range(NBATCH)]

            szp_all = accp.tile([C, B * NH], F32, tag="szp")
            zp_all = accp.tile([C, B], F32, tag="zp")
            szp = [szp_all[:, b * NH:(b + 1) * NH] for b in range(B)]
            zp = [zp_all[:, b:b + 1] for b in range(B)]

            p_tiles = {}
            GPB = NGRP // B                         # groups per batch (4)

            def emit_group_TL(g):
                """Transposes + logit matmuls + exp for chunk group g."""
                b = g // GPB
                lp = lpp.tile([C, KG * 2 * NH], F32, tag="lp")
                for jb in range(KG // TB):          # TB-batches per group
                    bi = g * (KG // TB) + jb        # global batch index
                    xtp = xtpp.tile([C, TB * CHP], BF16, tag="xtp")
                    for j in range(TB):
                        ch = g * KG + jb * TB + j
                        eo, pair = xchunk(ch)
                        nc.tensor.transpose(
                            xtp[:, j * CHP:(j + 1) * CHP], pair, ident_sb[:])
                        jj = jb * TB + j            # chunk index within group
                        for par in (0, 1):
                            col = jj * 2 * NH + par * NH
                            for hl in (0, 1):
                                nc.tensor.matmul(
                                    lp[:, col:col + NH],
                                    eo[par], aw_sb[:, b, hl, :],
                                    start=(hl == 0), stop=(hl == 1))
                    # PSUM -> SBUF copy of the transposed batch (DVE/ACT mix)
                    if bi % 4 == 3:
                        nc.scalar.copy(xts[bi][:], xtp[:])
                    else:
                        nc.vector.tensor_copy(xts[bi][:], xtp[:])
                pt = ppool.tile([C, KG * 2 * NH], FP8P, name=f"pt{g}", tag=f"pt{g}")
                nc.scalar.activation(pt[:], lp[:],
                                     mybir.ActivationFunctionType.Exp,
                                     bias=bias_sb[:])
                p_tiles[g] = pt

            outsb = consts.tile([C, B * 5], F32, tag="outsb")

            def emit_group_S(g):
                """Weighted-sum + Z matmuls for chunk group g; ship b's
                partials as soon as its accumulation closes."""
                b = g // GPB
                pt = p_tiles.pop(g)
                for jj in range(KG):
                    ch = g * KG + jj
                    bi, j = divmod(ch, TB)
                    x8t = xts[bi][:].bitcast(FP8X)
                    base = j * CH
                    for par in (0, 1):
                        first = (g == GPB * b and jj == 0 and par == 0)
                        last = (g == GPB * b + GPB - 1 and jj == KG - 1 and par == 1)
                        nc.tensor.matmul(
                            szp[b][:], x8t[:, base + par:base + CH:2],
                            pt[:, jj * 2 * NH + par * NH: jj * 2 * NH + (par + 1) * NH],
                            start=first, stop=last)
                nc.tensor.matmul(
                    zp[b][:], pt[:, 0:KG * 2 * NH], ones8[:],
                    start=(g == GPB * b), stop=(g == GPB * b + GPB - 1))
                if g == GPB * b + GPB - 1:
                    nc.vector.tensor_copy(outsb[:, b * 5:b * 5 + NH], szp[b][:])
                    nc.vector.tensor_copy(outsb[:, b * 5 + NH:b * 5 + 5], zp[b][:])
                    nc.gpsimd.dma_start(szout[:, b * 5:(b + 1) * 5],
                                        outsb[:, b * 5:(b + 1) * 5])

            emit_group_TL(0)
            emit_group_TL(1)
            for g in range(NGRP):
                if g + 2 < NGRP:
                    emit_group_TL(g + 2)
                emit_group_S(g)

    if waitfix:
        _split_excess_waits(nc)
    return nc


_NC_CACHE = {}


def _get_nc():
    if "nc" not in _NC_CACHE:
        _NC_CACHE["nc"] = _build_nc()
    return _NC_CACHE["nc"]


def _host_prep(diff_spatial, evolution_feat, ln_g, ln_b, gn_g, Wq, bq, Wk):
    """Exact (fp64) GroupNorm stats + folded logit coefficients A, split
    into bf16 hi/lo planes. Also the fp8 x in [C, B*N] layout, zero bytes
    dithered to the smallest denormal so bf16-pair views stay normal."""
    xf = diff_spatial.reshape(B, C, N)
    xg = xf.reshape(B, G, GS, N)
    mu = xg.mean(axis=(2, 3), dtype=np.float64)           # (B, G)
    ex2 = np.einsum("bgcn,bgcn->bg", xg, xg, dtype=np.float64) / (GS * N)
    var = ex2 - mu * mu
    rstd = 1.0 / np.sqrt(var + EPS)                        # (B, G)

    e = evolution_feat.astype(np.float64)
    emu = e.mean(axis=-1, keepdims=True)
    evar = e.var(axis=-1, keepdims=True)
    e = (e - emu) / np.sqrt(evar + EPS) * ln_g.astype(np.float64) + ln_b.astype(np.float64)
    q = e @ Wq.T.astype(np.float64) + bq.astype(np.float64)
    q = q.reshape(B, NH, HD)
    M = np.einsum("bhd,hdc->bhc", q, Wk.astype(np.float64).reshape(NH, HD, C))
    cg = np.arange(C) // GS
    A = (M * gn_g.astype(np.float64)[None, None, :] * (HD ** -0.5)
         * rstd[:, cg][:, None, :])                        # (B, NH, C)

    A_hi = A.astype(ml_dtypes.bfloat16)
    A_lo = (A - A_hi.astype(np.float64)).astype(ml_dtypes.bfloat16)
    aw = np.empty((C, B, 2, NH), ml_dtypes.bfloat16)
    aw[:, :, 0, :] = A_hi.transpose(2, 0, 1)
    aw[:, :, 1, :] = A_lo.transpose(2, 0, 1)

    x8 = np.ascontiguousarray(xf.transpose(1, 0, 2)).astype(ml_dtypes.float8_e3m4)
    v = x8.view(np.uint8)
    zero = (v & 0x7F) == 0
    v[zero] |= 1                                           # +-min denormal

    return x8, aw, mu, rstd


def kernel(diff_spatial, evolution_feat, ln_g, ln_b, gn_g, gn_b,
           Wq, bq, Wk, bk, Wv, bv, Wo, bo):
    nc = _get_nc()
    x8, aw, mu, rstd = _host_prep(
        np.asarray(diff_spatial, np.float32).reshape(B, C, N),
        np.asarray(evolution_feat, np.float32),
        np.asarray(ln_g, np.float32), np.asarray(ln_b, np.float32),
        np.asarray(gn_g, np.float32), np.asarray(Wq, np.float32),
        np.asarray(bq, np.float32), np.asarray(Wk, np.float32))

    identv = np.eye(C, dtype=np.float32).astype(ml_dtypes.bfloat16)
    in_maps = []
    for i in range(NCORES):
        xc = np.ascontiguousarray(x8[:, :, i * NS:(i + 1) * NS]).reshape(C, B * NS)
        in_maps.append({"x": xc, "aw": aw, "ident": identv})
    res = run_bass_kernel_spmd(nc, in_maps, list(range(NCORES)))
    return _host_finish(res.results, mu, rstd, gn_g, gn_b, Wv, bv, Wo, bo)


def _host_finish(results, mu, rstd, gn_g, gn_b, Wv, bv, Wo, bo):
    s_tot = np.zeros((B, NH, C), np.float64)
    z_tot = np.zeros((B, NH), np.float64)
    rr = np.arange(C)
    for r in results:
        o = r["szout"].astype(np.float64)                  # (C, B*5)
        for b in range(B):
            s_tot[b] += o[:, b * 5:b * 5 + NH].T           # (NH, C)
            zcol = o[:, b * 5 + NH]
            for h in range(NH):
                z_tot[b, h] += zcol[rr % NH == h].sum()

    cg = np.arange(C) // GS
    a = rstd[:, cg] * np.asarray(gn_g, np.float64)[None, :]
    d = np.asarray(gn_b, np.float64)[None, :] - mu[:, cg] * a
    y = a[:, None, :] * (s_tot / z_tot[:, :, None]) + d[:, None, :]

    Wvr = np.asarray(Wv, np.float64).reshape(NH, HD, C)
    o1 = np.einsum("hdc,bhc->bhd", Wvr, y).reshape(B, C) + np.asarray(bv, np.float64)
    out = o1 @ np.asarray(Wo, np.float64).T + np.asarray(bo, np.float64)
    return out.astype(np.float32)
